# revision 1
# baseline (speedup 1.0000x reference)
"""DenseDilatedKnnGraph Trainium2 kernel.

Problem: x (2, 256, 8192, 1) fp32. L2-normalize over channels, pairwise
euclidean distances per batch, ordered top-18 nearest neighbors per row,
output even-ranked neighbor indices + center indices: (2, 2, 8192, 9) int32.

Device strategy (8 NeuronCores, SPMD, no collectives):
  - core c handles batch c//4, query rows (c%4)*2048 ... +2048.
  - inputs per core: xb = x[batch] as [256, 8192] (full batch, rhs),
    xq = its 2048 query columns [256, 2048] (lhsT). Both normalized on
    device with identical instruction sequences -> bitwise-consistent.
  - score[i, j] = dot(xn_i, xn_j) via fp32 PE matmul (PSUM accumulate over
    two 128-row K chunks). Descending score == ascending distance.
  - top-k per 128-row tile: per-512-column PSUM tile, DVE max8 + max_index
    extract each chunk's top-8 values + local indices directly from PSUM
    (no SBUF score materialization). The 256 candidates per row are merged
    with max8 + match_replace8 into the ordered top-24 values; max_index
    over the candidate array gives each rank's candidate position
    (duplicate values get successive occurrences, matching jax.lax.top_k's
    smaller-index-first tie-break).
  - host: candidate-position -> global-index lookup, reshape, dilation
    slice, audit (candidate-coverage certificate + duplicate-index +
    finiteness checks), exact vectorized numpy recompute of flagged rows.
"""

import numpy as np

import concourse.mybir as mybir
import concourse.tile as tile
from concourse import bacc
from concourse.bass_utils import run_bass_kernel_spmd

F32 = mybir.dt.float32
U32 = mybir.dt.uint32

N_CORES = 8
B, C, N = 2, 256, 8192
RPC = N * B // N_CORES  # 2048 query rows per core
P = 128
KO = C // P             # 2 contraction chunks
RT = RPC // P           # 16 row tiles per core
CC = 512                # matmul column chunk (one PSUM bank fp32)
NCC = N // CC           # 16
CH = 512                # candidate chunk width
NCH = N // CH           # 32
NCAND = NCH * 8         # 256
KT = 18                 # k_total = K * DILATION
DIL = 2
KOUT = 9
NEG = -3.0e38

_CACHE = {}


def _normalize(nc, tc, pool, ps_pool, x_sb, n_cols, ones_sb, scratch_dram, tag,
               chunks=None):
    """In-place L2-normalize the columns of x_sb ([P, KO, n_cols], C on
    partitions), fully pipelined per 512-column chunk. Identical instruction
    sequence per column regardless of n_cols so xq columns match their xb
    counterparts bitwise."""
    if chunks is None:
        chunks = range(n_cols // CC)
    for cc in chunks:
        x2 = pool.tile([P, KO, CC], F32, name=f"x2_{tag}_{cc}", tag="x2")
        nc.scalar.square(x2, x_sb[:, cc])
        ps_s = ps_pool.tile([P, 4], F32, name=f"ps_s_{tag}_{cc}", tag="ps_s")
        for m in range(4):
            for ko in range(KO):
                nc.tensor.matmul(
                    ps_s[:, m:m + 1],
                    x2[:, ko, m * P:(m + 1) * P],
                    ones_sb,
                    start=(ko == 0),
                    stop=(ko == KO - 1),
                )
        s_cc = pool.tile([P, 4], F32, name=f"s_{tag}_{cc}", tag="s_cc")
        # match reference's x / max(norm, 1e-12): clamp before rsqrt so
        # zero-norm columns stay finite
        nc.vector.tensor_scalar_max(s_cc, ps_s, 1e-24)
        nc.scalar.sqrt(s_cc, s_cc)
        inv_cc = pool.tile([P, 4], F32, name=f"inv_{tag}_{cc}", tag="inv_cc")
        nc.vector.reciprocal(inv_cc, s_cc)
        # bounce to dram transposed (flat index = column index), then
        # broadcast-read a contiguous [1, CC] slice
        nc.sync.dma_start(
            scratch_dram[:].rearrange("(f p) -> p f", p=P)[:, cc * 4:(cc + 1) * 4],
            inv_cc)
        invb = pool.tile([P, CC], F32, name=f"invb_{tag}_{cc}", tag="invb")
        src = (
            scratch_dram[:][cc * CC:(cc + 1) * CC][None, :]
            .to_broadcast([P, CC])
        )
        nc.sync.dma_start(invb, src)
        nc.vector.tensor_tensor(
            x_sb[:, cc],
            x_sb[:, cc],
            invb[:, None, :].to_broadcast([P, KO, CC]),
            mybir.AluOpType.mult,
        )


def _build():
    nc = bacc.Bacc()
    xb_d = nc.declare_dram_parameter("xb", [C, N], F32, isOutput=False)
    xq_d = nc.declare_dram_parameter("xq", [C, RPC], F32, isOutput=False)
    o_p24 = nc.declare_dram_parameter("o_p24", [RT, P, 24], U32, isOutput=True)
    o_val = nc.declare_dram_parameter("o_val", [RT, P, 24], F32, isOutput=True)
    o_cv = nc.declare_dram_parameter("o_cv", [RT, P, NCAND], F32, isOutput=True)
    o_gi = nc.declare_dram_parameter("o_gi", [RT, P, NCAND], U32, isOutput=True)
    scr_b = nc.dram_tensor("scr_b", [4 * NCC * P], F32)
    scr_q = nc.dram_tensor("scr_q", [4 * (RPC // CC) * P], F32)

    with tile.TileContext(nc) as tc:
        with (
            tc.tile_pool(name="big", bufs=1) as big,
            tc.tile_pool(name="work", bufs=2) as work,
            tc.tile_pool(name="ps", bufs=6, space="PSUM") as ps,
        ):
            ones_sb = big.tile([P, 1], F32)
            nc.vector.memset(ones_sb, 1.0)
            # offs[p, c] = CH * (c // 8): candidate -> chunk base offset
            offs = big.tile([P, NCAND], U32)
            nc.gpsimd.iota(
                offs.rearrange("p (i j) -> p i j", i=NCH),
                pattern=[[CH, NCH], [0, 8]],
                base=0,
                channel_multiplier=0,
            )

            # chunk-major layout [P, chunk, KO, CC]: each 512-column chunk is
            # byte-contiguous per partition, so subtile dependency ranges do
            # not overlap across chunks. Emit each chunk's input DMA
            # immediately followed by its normalization so the tiny bounce
            # DMAs queue right behind their own chunk's input transfer
            # instead of behind every input DMA.
            qs = [nc.sync, nc.scalar]
            xq = big.tile([P, RPC // CC, KO, CC], F32)
            xb = big.tile([P, N // CC, KO, CC], F32)
            with (
                tc.tile_pool(name="norm", bufs=2) as normp,
                tc.tile_pool(name="ps_n", bufs=2, space="PSUM") as ps_n,
            ):
                order = []
                for cc in range(RPC // CC):
                    order.append(("q", cc))
                    order.append(("b", cc))
                order += [("b", cc) for cc in range(RPC // CC, N // CC)]
                for i, (which, cc) in enumerate(order):
                    x_sb, xd, scr, n_cols = (
                        (xq, xq_d, scr_q, RPC) if which == "q"
                        else (xb, xb_d, scr_b, N))
                    qs[i % 2].dma_start(
                        x_sb[:, cc],
                        xd[:, cc * CC:(cc + 1) * CC].rearrange(
                            "(ko p) n -> p ko n", p=P))
                    _normalize(nc, tc, normp, ps_n, x_sb, n_cols, ones_sb,
                               scr, which, chunks=[cc])

            for t in range(RT):
                cv = work.tile([P, NCAND], F32, name=f"cv_{t}", tag="cv")
                li = work.tile([P, NCAND], U32, name=f"li_{t}", tag="li")
                for cc in range(NCC):
                    ps_t = ps.tile([P, CC], F32, name=f"ps_{t}_{cc}", tag="ps_sc")
                    for ko in range(KO):
                        nc.tensor.matmul(
                            ps_t,
                            xq[:, t // 4, ko, (t % 4) * P:(t % 4 + 1) * P],
                            xb[:, cc, ko],
                            start=(ko == 0),
                            stop=(ko == KO - 1),
                        )
                    # candidate extraction straight from PSUM (CH == CC)
                    nc.vector.max(
                        out=cv[:, cc * 8:(cc + 1) * 8], in_=ps_t)
                    nc.vector.max_index(
                        li[:, cc * 8:(cc + 1) * 8], cv[:, cc * 8:(cc + 1) * 8],
                        ps_t)
                gi = work.tile([P, NCAND], U32, name=f"gi_{t}", tag="gi")
                nc.vector.tensor_tensor(gi, li, offs, mybir.AluOpType.add)

                v24 = work.tile([P, 24], F32, name=f"v24_{t}", tag="v24")
                p24 = work.tile([P, 24], U32, name=f"p24_{t}", tag="p24")
                mv0 = work.tile([P, NCAND], F32, name=f"mv0_{t}", tag="mv0")
                mv1 = work.tile([P, NCAND], F32, name=f"mv1_{t}", tag="mv1")
                nc.vector.max(out=v24[:, 0:8], in_=cv)
                nc.vector.match_replace(
                    out=mv0, in_to_replace=v24[:, 0:8], in_values=cv, imm_value=NEG)
                nc.vector.max(out=v24[:, 8:16], in_=mv0)
                nc.vector.match_replace(
                    out=mv1, in_to_replace=v24[:, 8:16], in_values=mv0, imm_value=NEG)
                nc.vector.max(out=v24[:, 16:24], in_=mv1)
                for g in range(3):
                    nc.vector.max_index(
                        p24[:, g * 8:(g + 1) * 8], v24[:, g * 8:(g + 1) * 8], cv)

                nc.sync.dma_start(o_p24[:][t], p24)
                nc.sync.dma_start(o_val[:][t], v24)
                nc.sync.dma_start(o_cv[:][t], cv)
                nc.sync.dma_start(o_gi[:][t], gi)

    nc.finalize()
    return nc


def _get_nc():
    if "nc" not in _CACHE:
        _CACHE["nc"] = _build()
    return _CACHE["nc"]


def _reference_rows(xn, sq, b, rows):
    """Exact reference ordering for a set of rows of one batch (numpy fp32,
    matches jax semantics: dist ascending, ties -> smaller index first)."""
    d2 = sq[b][None, :] + sq[b][rows, None] - 2.0 * (xn[b][rows] @ xn[b].T)
    dist = np.sqrt(np.maximum(d2, 0.0), dtype=np.float32)
    # stable argsort by distance == top_k tie-break (smaller index first)
    order = np.argsort(dist, axis=1, kind="stable")
    return order[:, :KT]


def kernel(x, relative_pos=None, **_unused):
    x = np.ascontiguousarray(np.asarray(x), dtype=np.float32)
    assert x.shape == (B, C, N, 1), x.shape

    nc = _get_nc()
    xmat = x[..., 0]  # (B, C, N)
    in_maps = []
    for c in range(N_CORES):
        b = c // (N_CORES // B)
        r0 = (c % (N_CORES // B)) * RPC
        in_maps.append({
            "xb": np.ascontiguousarray(xmat[b]),
            "xq": np.ascontiguousarray(xmat[b][:, r0:r0 + RPC]),
        })
    res = run_bass_kernel_spmd(nc, in_maps, core_ids=list(range(N_CORES)))

    p24 = np.zeros((B, N, 24), np.int64)
    val = np.zeros((B, N, 24), np.float32)
    cv8 = np.zeros((B, N, NCH), np.float32)
    gi = np.zeros((B, N, NCAND), np.int64)
    for c in range(N_CORES):
        b = c // (N_CORES // B)
        r0 = (c % (N_CORES // B)) * RPC
        r = res.results[c]
        p24[b, r0:r0 + RPC] = r["o_p24"].reshape(RPC, 24).astype(np.int64)
        val[b, r0:r0 + RPC] = r["o_val"].reshape(RPC, 24)
        cv8[b, r0:r0 + RPC] = r["o_cv"].reshape(RPC, NCAND)[:, 7::8]
        gi[b, r0:r0 + RPC] = r["o_gi"].reshape(RPC, NCAND).astype(np.int64)

    # candidate position -> global column index (pure indexing)
    bad_pos = (p24[:, :, :KT] < 0) | (p24[:, :, :KT] >= NCAND)
    nn = np.take_along_axis(gi, np.clip(p24[:, :, :KT], 0, NCAND - 1), axis=2)

    # ---- audit ----
    t18 = val[:, :, KT - 1]
    bad_cert = (cv8 >= t18[:, :, None]).any(axis=2)
    srt = np.sort(nn, axis=2)
    bad_dup = (np.diff(srt, axis=2) == 0).any(axis=2)
    bad_inval = (nn < 0).any(axis=2) | (nn >= N).any(axis=2) | bad_pos.any(axis=2)
    bad_fin = ~np.isfinite(val).all(axis=2) | ~np.isfinite(cv8).all(axis=2)
    flagged = np.argwhere(bad_cert | bad_dup | bad_inval | bad_fin)
    kernel.n_flagged = len(flagged)
    if len(flagged):
        xt = xmat.transpose(0, 2, 1)  # (B, N, C)
        norm = np.sqrt((xt * xt).sum(-1, dtype=np.float32), dtype=np.float32)
        xn = xt / np.maximum(norm, 1e-12)[..., None]
        sq = (xn * xn).sum(-1, dtype=np.float32)
        for b in range(B):
            rows = flagged[flagged[:, 0] == b][:, 1]
            if len(rows):
                nn[b, rows] = _reference_rows(xn, sq, b, rows)

    center = np.broadcast_to(np.arange(N, dtype=np.int64)[None, :, None], (B, N, KT))
    edge = np.stack((nn, center), axis=0)        # (2, B, N, 18)
    return edge[:, :, :, ::DIL].astype(np.int32)  # (2, 2, 8192, 9)


if __name__ == "__main__":
    xs = np.random.default_rng(0).standard_normal((B, C, N, 1), dtype=np.float32)
    out = kernel(xs, np.zeros(1, np.float32))
    print(out.shape, out.dtype)



# revision 6
# speedup vs baseline: 1.8126x; 1.8126x over previous
"""DenseDilatedKnnGraph Trainium2 kernel — mask+moment extraction.

Device (8 cores SPMD, core c: batch c//4, query rows (c%4)*2048..+2048):
  - normalize columns of xb [256, 8192] and xq [256, 2048] on device
    (L2 over C), emitting bf16 copies (scale pass writes bf16).
  - transposed score tiles: lhsT = candidate block [C128, 128], rhs =
    query chunk [C128, 512] -> PSUM [128 cand, 512 query], bf16 matmuls
    (1 cycle/row vs 4 for fp32: Tensor 437us -> 109us).
  - threshold masks: mask = (score > TAU) ? 1 : 0 as fp16, produced
    straight from PSUM by DVE/Pool tensor_scalar (is_gt), split across
    both engines.
  - moment matmuls: Wm[128, 24] x mask -> per sub-block-of-16
    {count, sum(idx), sum(idx^2)} accumulated exactly in fp32 PSUM;
    4 candidate blocks packed per PSUM tile (partition offsets 0/32/64/96),
    DMA'd straight from PSUM to DRAM.
  - no DVE max8/max_index scans (1x-rate ops) anywhere: the only
    full-resolution passes are the matmuls (PE) and one is_gt pass
    (DVE+Pool), so every engine stays under the PE's ~110us.

Host: solve the integer moment systems (c==1 direct, c==2 via
sum/sum-of-squares), mini-rescore the rare sub-blocks with c>=3 or
inconsistent moments, exact-rescore all candidates in fp32 (reference
math), order by (dist, idx), certificate s18 > TAU + delta, full
reference recompute for flagged rows, dilate."""

import numpy as np

import concourse.mybir as mybir
import concourse.tile as tile
from concourse import bacc
from concourse.bass_utils import run_bass_kernel_spmd

F32 = mybir.dt.float32
F16 = mybir.dt.float16
BF16 = mybir.dt.bfloat16

N_CORES = 8
B, C, N = 2, 256, 8192
RPC = N * B // N_CORES  # 2048 query rows per core
P = 128
KO = C // P             # 2 contraction chunks
CC = 512                # query chunk width (matmul moving width)
NQC = RPC // CC         # 4 query chunks
NB = N // P             # 64 candidate blocks
SUB = 16                # moment sub-block size
NSUB = P // SUB         # 8 subs per block
NMCOL = 3 * NSUB        # 24 moment rows per block
BPG = 4                 # candidate blocks per moment PSUM tile
NG = NB // BPG          # 16 groups
TAU = 0.159
DELTA = 2e-3            # device(bf16) vs host(fp32) score-noise bound
KT = 18                 # k_total = K * DILATION
DIL = 2

_CACHE = {}


def _normalize(nc, pool, ps_pool, x_sb, out_sb, ones_sb, scratch_dram, tag,
               chunks):
    """L2-normalize columns of x_sb ([P, nch, KO, CC], C on partitions) into
    bf16 out_sb (same layout), per 512-column chunk."""
    for cc in chunks:
        x2 = pool.tile([P, KO, CC], F32, name=f"x2_{tag}_{cc}", tag="x2")
        nc.scalar.square(x2, x_sb[:, cc])
        ps_s = ps_pool.tile([P, 4], F32, name=f"ps_s_{tag}_{cc}", tag="ps_s")
        for m in range(4):
            for ko in range(KO):
                nc.tensor.matmul(
                    ps_s[:, m:m + 1],
                    x2[:, ko, m * P:(m + 1) * P],
                    ones_sb,
                    start=(ko == 0),
                    stop=(ko == KO - 1),
                )
        s_cc = pool.tile([P, 4], F32, name=f"s_{tag}_{cc}", tag="s_cc")
        nc.vector.tensor_scalar_max(s_cc, ps_s, 1e-24)
        nc.scalar.sqrt(s_cc, s_cc)
        inv_cc = pool.tile([P, 4], F32, name=f"inv_{tag}_{cc}", tag="inv_cc")
        nc.vector.reciprocal(inv_cc, s_cc)
        # bounce to dram transposed (flat index = column index), then
        # broadcast-read a contiguous [1, CC] slice
        nc.sync.dma_start(
            scratch_dram[:].rearrange("(f p) -> p f", p=P)[:, cc * 4:(cc + 1) * 4],
            inv_cc)
        invb = pool.tile([P, CC], F32, name=f"invb_{tag}_{cc}", tag="invb")
        src = (
            scratch_dram[:][cc * CC:(cc + 1) * CC][None, :]
            .to_broadcast([P, CC])
        )
        nc.sync.dma_start(invb, src)
        nc.vector.tensor_tensor(
            out_sb[:, cc],
            x_sb[:, cc],
            invb[:, None, :].to_broadcast([P, KO, CC]),
            mybir.AluOpType.mult,
        )


def _build():
    nc = bacc.Bacc()
    xb_d = nc.declare_dram_parameter("xb", [C, N], F32, isOutput=False)
    xq_d = nc.declare_dram_parameter("xq", [C, RPC], F32, isOutput=False)
    wm_d = nc.declare_dram_parameter("wm", [P, NMCOL], F16, isOutput=False)
    o_mom = nc.declare_dram_parameter("o_mom", [NQC, 2 * NG, P, CC], F32,
                                      isOutput=True)
    scr_b = nc.dram_tensor("scr_b", [N], F32)
    scr_q = nc.dram_tensor("scr_q", [RPC], F32)

    with tile.TileContext(nc) as tc:
        with (
            tc.tile_pool(name="big", bufs=1) as big,
            tc.tile_pool(name="work", bufs=3) as work,
            tc.tile_pool(name="ps_sc", bufs=2, space="PSUM") as ps_sc,
            tc.tile_pool(name="ps_mom", bufs=2, space="PSUM") as ps_mom,
        ):
            ones_sb = big.tile([P, 1], F32)
            nc.vector.memset(ones_sb, 1.0)
            taub = big.tile([P, 1], F32)
            nc.vector.memset(taub, -TAU)
            wm_sb = big.tile([P, NMCOL], F16)
            nc.sync.dma_start(wm_sb, wm_d[:])

            xb = big.tile([P, N // CC, KO, CC], F32)
            xq = big.tile([P, NQC, KO, CC], F32)
            xbn = big.tile([P, N // CC, KO, CC], BF16)
            xqn = big.tile([P, NQC, KO, CC], BF16)
            qs = [nc.sync, nc.scalar]
            with (
                tc.tile_pool(name="norm", bufs=2) as normp,
                tc.tile_pool(name="ps_n", bufs=2, space="PSUM") as ps_n,
            ):
                order = []
                for cc in range(NQC):
                    order.append(("q", cc))
                    order.append(("b", cc))
                order += [("b", cc) for cc in range(NQC, N // CC)]
                for i, (which, cc) in enumerate(order):
                    x_sb, x_out, xd, scr = (
                        (xq, xqn, xq_d, scr_q) if which == "q"
                        else (xb, xbn, xb_d, scr_b))
                    qs[i % 2].dma_start(
                        x_sb[:, cc],
                        xd[:, cc * CC:(cc + 1) * CC].rearrange(
                            "(ko p) n -> p ko n", p=P))
                    _normalize(nc, normp, ps_n, x_sb, x_out, ones_sb,
                               scr, which, chunks=[cc])

            for qc in range(NQC):
                for g in range(NG):
                    for h in range(2):  # one 2-block score tile per pair
                        sc = ps_sc.tile([P, 2 * CC], F32,
                                        name=f"sc_{qc}_{g}_{h}", tag="sc")
                        for jj in range(2):
                            bb = g * BPG + h * 2 + jj
                            for ko in range(KO):
                                nc.tensor.matmul(
                                    sc[:, jj * CC:(jj + 1) * CC],
                                    xbn[:, bb // 4, ko,
                                        (bb % 4) * P:(bb % 4 + 1) * P],
                                    xqn[:, qc, ko],
                                    start=(ko == 0),
                                    stop=(ko == KO - 1),
                                )
                        # Act drains every score tile: mask = sign(s - TAU)
                        # in {-1, +1} fp16 (GPSIMD cannot access PSUM; DVE
                        # is busy with moment drains)
                        mask = work.tile([P, 2 * CC], F16,
                                         name=f"mk_{qc}_{g}_{h}", tag="mask")
                        nc.scalar.sign(mask, sc, bias=taub)
                        # moments of the two blocks at partition offsets 0/64
                        mom = ps_mom.tile([P, CC], F32,
                                          name=f"mom_{qc}_{g}_{h}", tag="mom")
                        for jj in range(2):
                            nc.tensor.matmul(
                                mom[64 * jj:64 * jj + NMCOL, :], wm_sb,
                                mask[:, jj * CC:(jj + 1) * CC],
                                start=True, stop=True)
                        # PSUM can't be DMA'd directly: drain via DVE
                        mom_sb = work.tile([P, CC], F32,
                                           name=f"mo_{qc}_{g}_{h}",
                                           tag="mom_sb")
                        nc.vector.tensor_copy(mom_sb, mom)
                        qs[h].dma_start(o_mom[:][qc, 2 * g + h], mom_sb)

    nc.finalize()
    return nc


def _get_nc():
    if "nc" not in _CACHE:
        _CACHE["nc"] = _build()
    return _CACHE["nc"]


def _make_wm():
    wm = np.zeros((P, NMCOL), np.float16)
    for p in range(P):
        s, l = p // SUB, p % SUB
        wm[p, 3 * s + 0] = 1.0
        wm[p, 3 * s + 1] = l
        wm[p, 3 * s + 2] = l * l
    return wm


def make_in_maps(xmat):
    wm = _make_wm()
    in_maps = []
    for c in range(N_CORES):
        b = c // (N_CORES // B)
        r0 = (c % (N_CORES // B)) * RPC
        in_maps.append({
            "xb": np.ascontiguousarray(xmat[b]),
            "xq": np.ascontiguousarray(xmat[b][:, r0:r0 + RPC]),
            "wm": wm,
        })
    return in_maps


def _reference_rows(xn, sq, b, rows):
    """Exact reference ordering for a set of rows of one batch."""
    d2 = sq[b][None, :] + sq[b][rows, None] - 2.0 * (xn[b][rows] @ xn[b].T)
    dist = np.sqrt(np.maximum(d2, 0.0), dtype=np.float32)
    order = np.argsort(dist, axis=1, kind="stable")
    return order[:, :KT]


def _pair_scores(xn_b, sq_b, rows, cands):
    """Exact fp32 (dist, score) for candidate pairs, chunked."""
    n = len(rows)
    dist = np.empty(n, np.float32)
    s = np.empty(n, np.float32)
    CH = 200_000
    for i in range(0, n, CH):
        r = rows[i:i + CH]
        cix = cands[i:i + CH]
        sc = np.einsum("pc,pc->p", xn_b[r], xn_b[cix], dtype=np.float32)
        d2 = sq_b[r] + sq_b[cix] - 2.0 * sc
        dist[i:i + CH] = np.sqrt(np.maximum(d2, 0.0), dtype=np.float32)
        s[i:i + CH] = sc
    return dist, s


def kernel(x, relative_pos=None, **_unused):
    x = np.ascontiguousarray(np.asarray(x), dtype=np.float32)
    assert x.shape == (B, C, N, 1), x.shape

    nc = _get_nc()
    xmat = x[..., 0]  # (B, C, N)
    res = run_bass_kernel_spmd(nc, make_in_maps(xmat),
                               core_ids=list(range(N_CORES)))

    # (B, N_rows, 512 subs, 3 moments)
    M = np.zeros((B, N, N // SUB, 3), np.float32)
    for c in range(N_CORES):
        b = c // (N_CORES // B)
        r0 = (c % (N_CORES // B)) * RPC
        m = res.results[c]["o_mom"].reshape(NQC, 2 * NG, 2, 64, CC)
        m = m[:, :, :, :NMCOL, :].reshape(NQC, 2 * NG, 2, NSUB, 3, CC)
        # [qc, pair, jj, s, mom, q] -> [qc, q, pair, jj, s, mom]
        m = m.transpose(0, 5, 1, 2, 3, 4).reshape(RPC, N // SUB, 3)
        M[b, r0:r0 + RPC] = m

    xt = xmat.transpose(0, 2, 1)
    norm = np.sqrt((xt * xt).sum(-1, dtype=np.float32), dtype=np.float32)
    xn = (xt / np.maximum(norm, 1e-12)[..., None]).astype(np.float32)
    sq = (xn * xn).sum(-1, dtype=np.float32)

    # all masks are Act sign masks (+-1): S_above(w) = (m + T_w) / 2
    m0 = np.rint(M[..., 0]).astype(np.int64)
    m1 = np.rint(M[..., 1]).astype(np.int64)
    m2 = np.rint(M[..., 2]).astype(np.int64)
    exact0 = ((np.abs(M[..., 0] - m0) < 1e-3)
              & (np.abs(M[..., 1] - m1) < 1e-3)
              & (np.abs(M[..., 2] - m2) < 1e-3))
    T1, T2 = 120, 1240  # sum l, sum l^2 over a sub-block
    c_ = (m0 + SUB) >> 1
    s1 = (m1 + T1) >> 1
    s2 = (m2 + T2) >> 1
    par_ok = ((((m0 + SUB) & 1) == 0) & (((m1 + T1) & 1) == 0)
              & (((m2 + T2) & 1) == 0))
    base_ok = exact0 & par_ok
    ok0 = (c_ == 0) & base_ok & (s1 == 0) & (s2 == 0)
    ok1 = ((c_ == 1) & base_ok & (s1 >= 0) & (s1 < SUB) & (s2 == s1 * s1))
    disc = 2 * s2 - s1 * s1
    ri = np.rint(np.sqrt(np.maximum(disc, 0))).astype(np.int64)
    a = (s1 + ri) >> 1
    b2 = (s1 - ri) >> 1
    ok2 = ((c_ == 2) & base_ok & (disc > 0) & (ri * ri == disc)
           & (((s1 + ri) & 1) == 0) & (a < SUB) & (b2 >= 0) & (a != b2)
           & (a * a + b2 * b2 == s2))
    flag_sub = ~(ok0 | ok1 | ok2)

    nn = np.zeros((B, N, KT), np.int64)
    flag_rows = [None] * B
    n_flagged = 0
    for b in range(B):
        rows_l = []
        cand_l = []
        rr1, ss1 = np.nonzero(ok1[b])
        rows_l.append(rr1)
        cand_l.append(ss1 * SUB + s1[b][rr1, ss1])
        rr2, ss2 = np.nonzero(ok2[b])
        rows_l += [rr2, rr2]
        cand_l += [ss2 * SUB + a[b][rr2, ss2], ss2 * SUB + b2[b][rr2, ss2]]
        # mini-rescore flagged subs exactly (c>=3 or inconsistent moments)
        rf, sf = np.nonzero(flag_sub[b])
        if len(rf):
            qv = xn[b][rf]                                     # (F, C)
            cv = xn[b].reshape(N // SUB, SUB, C)[sf]           # (F, SUB, C)
            sc = np.einsum("fc,fkc->fk", qv, cv, dtype=np.float32)
            fr, fk = np.nonzero(sc > TAU - DELTA)
            rows_l.append(rf[fr])
            cand_l.append(sf[fr] * SUB + fk)
        rows = np.concatenate(rows_l)
        cands = np.concatenate(cand_l)

        dist, s = _pair_scores(xn[b], sq[b], rows, cands)
        order = np.lexsort((cands, dist, rows))
        rows_s = rows[order]
        cands_s = cands[order]
        s_s = s[order]
        starts = np.searchsorted(rows_s, np.arange(N))
        counts = np.diff(np.append(starts, len(rows_s)))
        rank = np.arange(len(rows_s)) - starts[rows_s]
        sel = rank < KT
        nn[b][rows_s[sel], rank[sel]] = cands_s[sel]
        s18 = np.full(N, -2.0, np.float32)
        at18 = rank == (KT - 1)
        s18[rows_s[at18]] = s_s[at18]
        bad = (counts < KT) | (s18 <= TAU + DELTA)
        flag_rows[b] = np.nonzero(bad)[0]
        n_flagged += len(flag_rows[b])

    kernel.n_flagged = n_flagged
    for b in range(B):
        if len(flag_rows[b]):
            nn[b][flag_rows[b]] = _reference_rows(xn, sq, b, flag_rows[b])

    center = np.broadcast_to(
        np.arange(N, dtype=np.int64)[None, :, None], (B, N, KT))
    edge = np.stack((nn, center), axis=0)          # (2, B, N, 18)
    return edge[:, :, :, ::DIL].astype(np.int32)   # (2, 2, 8192, 9)


if __name__ == "__main__":
    xs = np.random.default_rng(0).standard_normal((B, C, N, 1),
                                                  dtype=np.float32)
    out = kernel(xs, np.zeros(1, np.float32))
    print(out.shape, out.dtype, getattr(kernel, "n_flagged", None))
